# revision 5
# baseline (speedup 1.0000x reference)
"""BiMamba block on 8 Trainium2 NeuronCores.

Sharding: d_inner (2048) split 8 ways -> 256 channels/core, both branches,
both batch elems per core. Cross-core collectives: AllReduce for the x_proj
partial sums (reduction over d_inner), ReduceScatter for the output
projection (reduction over d_inner, output row-sharded).

Core layout choices:
- tokens t = b*2048 + l along the free dim (4096 per core); scans split per
  batch at chunk boundaries.
- SSM scan runs per "group": 128 partitions = 4 n-values x 32 channels
  (p = ln*32 + lc). 8 channel-blocks (cb) x 4 n-quarters (np) = 32 groups.
- dt/w replication across n via PE selector matmuls (PSUM), dA = exp via
  ScalarE with per-partition scale A, dB/scan on VectorE
  (tensor_tensor_scan does the recurrence), y-reduce over n via col-tiled
  accumulating matmuls into one PSUM bank, gating on VectorE.
- bwd branch runs in flipped time via reversed access patterns (no copies).
"""
import sys, os
sys.path.insert(0, '/opt/trn_rl_repo')
os.environ.setdefault("JAX_PLATFORMS", "cpu")

import numpy as np
from contextlib import ExitStack

import concourse.bass as bass
import concourse.tile as tile
from concourse import bacc, mybir
from concourse.bass_utils import run_bass_kernel_spmd

F32 = mybir.dt.float32
BF16 = mybir.dt.bfloat16
AF = mybir.ActivationFunctionType
ALU = mybir.AluOpType

# problem sizes
B, L, DM, DI, N, R, KC = 2, 2048, 1024, 2048, 16, 64, 4
NC = 8
CH = DI // NC            # 256 channels per core
TOK = B * L              # 4096
CK = 512                 # token chunk
NCK = TOK // CK          # 8
CPB = L // CK            # chunks per batch = 4
NCB = CH // 32           # 8 channel-blocks of 32
NP = N // 4              # 4 n-quarters
PADL = L + 6             # per-batch padded cols: [3 zero][L data][3 zero]

# engine flags
HC_ON_GPSIMD = True

_CACHE = {}


def build_program():
    nc = bacc.Bacc("TRN2", target_bir_lowering=False, debug=False,
                   num_devices=NC)

    ext = {}
    def ein(name, shape):
        ext[name] = nc.dram_tensor(name, list(shape), F32,
                                   kind="ExternalInput")
        return ext[name]

    uT = ein("uT", (DM, TOK))
    winT = ein("winT", (DM, 2 * CH))
    woutT = ein("woutT", (CH, DM))
    sel = ein("sel", (128, 4 * 128))
    sel32 = ein("sel32", (128, 32))
    for p in ("f", "b"):
        ein(f"{p}cw", (CH, KC))
        ein(f"{p}cbias", (CH, 1))
        ein(f"{p}xpT", (128, 2 * 96))
        ein(f"{p}dtwT", (R, CH))
        ein(f"{p}dtb", (CH, 1))
        ein(f"{p}acols", (128, 32))
        ein(f"{p}dvec", (CH, 1))

    out_slice = nc.dram_tensor("out_slice", [DM // NC, TOK], F32,
                               kind="ExternalOutput")

    sz_dram = nc.dram_tensor("sz_dram", [CH, TOK], F32)
    cc_in = nc.dram_tensor("cc_in", [192, TOK], F32)
    cc_out = nc.dram_tensor("cc_out", [192, TOK], F32, addr_space="Shared")
    out_cc_in = nc.dram_tensor("out_cc_in", [DM, TOK], F32)
    rs_out = nc.dram_tensor("rs_out", [DM // NC, TOK], F32)

    GROUPS = [list(range(NC))]

    with tile.TileContext(nc) as tc, ExitStack() as ctx:
        # ---- persistent pools
        wp = ctx.enter_context(tc.tile_pool(name="wp", bufs=1))
        big = ctx.enter_context(tc.tile_pool(name="big", bufs=1))

        sel_sb = wp.tile([128, 4 * 128], F32)
        nc.sync.dma_start(sel_sb[:], sel[:])
        sel32_sb = wp.tile([128, 32], F32)
        nc.sync.dma_start(sel32_sb[:], sel32[:])
        wout_sb = wp.tile([128, 2 * DM], F32)
        for k in range(2):
            nc.sync.dma_start(wout_sb[:, k * DM:(k + 1) * DM],
                              woutT[k * 128:(k + 1) * 128, :])

        br_w = {}
        for p in ("f", "b"):
            d = {}
            d["cw"] = wp.tile([128, 2 * KC], F32, name=f"{p}cw_sb")
            for ct in range(2):
                nc.sync.dma_start(d["cw"][:, ct * KC:(ct + 1) * KC],
                                  ext[f"{p}cw"][ct * 128:(ct + 1) * 128, :])
            for nm in ("cbias", "dtb", "dvec"):
                t_ = wp.tile([128, 2], F32, name=f"{p}{nm}_sb")
                for ct in range(2):
                    nc.sync.dma_start(
                        t_[:, ct:ct + 1],
                        ext[f"{p}{nm}"][ct * 128:(ct + 1) * 128, :])
                d[nm] = t_
            d["xpT"] = wp.tile([128, 2 * 96], F32, name=f"{p}xpT_sb")
            nc.sync.dma_start(d["xpT"][:], ext[f"{p}xpT"][:])
            d["dtwT"] = wp.tile([R, CH], F32, name=f"{p}dtwT_sb")
            nc.sync.dma_start(d["dtwT"][:], ext[f"{p}dtwT"][:])
            d["acols"] = wp.tile([128, 32], F32, name=f"{p}acols_sb")
            nc.sync.dma_start(d["acols"][:], ext[f"{p}acols"][:])
            br_w[p] = d

        # persistent activations: xc per branch per ct
        xc = {p: [big.tile([128, TOK], F32, name=f"xc{p}{ct}")
                  for ct in range(2)] for p in ("f", "b")}

        # ============ phase 1: in_proj + conv + x_proj partials ============
        with tc.tile_pool(name="ph1", bufs=1) as ph1, \
             tc.tile_pool(name="w1", bufs=2) as w1, \
             tc.tile_pool(name="ps1", bufs=2, space="PSUM") as ps1:

            win_sb = ph1.tile([128, 8 * 512], F32)
            for k in range(8):
                nc.sync.dma_start(win_sb[:, k * 512:(k + 1) * 512],
                                  winT[k * 128:(k + 1) * 128, :])
            x_pad = [ph1.tile([128, B * PADL], F32, name=f"xpad{ct}")
                     for ct in range(2)]
            for ct in range(2):
                for bb in range(B):
                    nc.vector.memset(
                        x_pad[ct][:, bb * PADL:bb * PADL + 3], 0.0)
                    nc.vector.memset(
                        x_pad[ct][:, bb * PADL + 3 + L:(bb + 1) * PADL], 0.0)

            def dcol(ckk):
                bb = ckk // CPB
                return bb * PADL + 3 + (ckk % CPB) * CK

            # in_proj
            for ck in range(NCK):
                ut = w1.tile([128, 8 * CK], F32, tag="ut")
                for k in range(8):
                    nc.sync.dma_start(ut[:, k * CK:(k + 1) * CK],
                                      uT[k * 128:(k + 1) * 128,
                                         ck * CK:(ck + 1) * CK])
                for mt in range(4):
                    pin = ps1.tile([128, CK], F32, tag="pa")
                    for k in range(8):
                        nc.tensor.matmul(
                            pin[:], win_sb[:, k * 512 + mt * 128:
                                           k * 512 + (mt + 1) * 128],
                            ut[:, k * CK:(k + 1) * CK],
                            start=(k == 0), stop=(k == 7))
                    if mt < 2:
                        c0 = dcol(ck)
                        nc.scalar.copy(x_pad[mt][:, c0:c0 + CK], pin[:])
                    else:
                        ct = mt - 2
                        sg = w1.tile([128, CK], F32, tag="sg")
                        nc.scalar.activation(sg[:], pin[:], AF.Sigmoid)
                        szt = w1.tile([128, CK], F32, tag="szt")
                        nc.vector.tensor_mul(szt[:], pin[:], sg[:])
                        nc.sync.dma_start(
                            sz_dram[ct * 128:(ct + 1) * 128,
                                    ck * CK:(ck + 1) * CK], szt[:])

            # conv + silu + x_proj partials, both branches
            for p in ("f", "b"):
                d = br_w[p]
                for ck in range(NCK):
                    bb = ck // CPB
                    cc = ck % CPB
                    for ct in range(2):
                        acc = None
                        for k in range(4):
                            if p == "f":
                                c0 = dcol(ck)
                                src = x_pad[ct][:, c0 - 3 + k:
                                                c0 - 3 + k + CK]
                            else:
                                c_hi = bb * PADL + 3 + (L - 1) - cc * CK \
                                    + 3 - k
                                src = x_pad[ct][:, c_hi - CK + 1:
                                                c_hi + 1][:, ::-1]
                            if k == 0:
                                acc = w1.tile([128, CK], F32, tag="acc0")
                                nc.vector.tensor_scalar_mul(
                                    acc[:], src,
                                    d["cw"][:, ct * KC:ct * KC + 1])
                            else:
                                acc2 = w1.tile([128, CK], F32, tag=f"acc{k}")
                                nc.vector.scalar_tensor_tensor(
                                    acc2[:], src,
                                    d["cw"][:, ct * KC + k:ct * KC + k + 1],
                                    acc[:], ALU.mult, ALU.add)
                                acc = acc2
                        sg = w1.tile([128, CK], F32, tag="sgc")
                        nc.scalar.activation(sg[:], acc[:], AF.Sigmoid,
                                             bias=d["cbias"][:, ct:ct + 1])
                        nc.vector.scalar_tensor_tensor(
                            xc[p][ct][:, ck * CK:(ck + 1) * CK], acc[:],
                            d["cbias"][:, ct:ct + 1], sg[:],
                            ALU.add, ALU.mult)
                    pxp = ps1.tile([96, CK], F32, tag="pxp")
                    for ct in range(2):
                        nc.tensor.matmul(
                            pxp[:], d["xpT"][:, ct * 96:(ct + 1) * 96],
                            xc[p][ct][:, ck * CK:(ck + 1) * CK],
                            start=(ct == 0), stop=(ct == 1))
                    pj = w1.tile([96, CK], F32, tag="pj")
                    nc.scalar.copy(pj[:], pxp[:])
                    row0 = 0 if p == "f" else 96
                    nc.sync.dma_start(
                        cc_in[row0:row0 + 96, ck * CK:(ck + 1) * CK], pj[:])

        # ============ AllReduce x_proj partials ============
        nc.gpsimd.collective_compute(
            "AllReduce", ALU.add, replica_groups=GROUPS,
            ins=[cc_in.ap()], outs=[cc_out.ap()])

        # ============ phase 2: scan + gate + out_proj ============
        with tc.tile_pool(name="ph2", bufs=1) as ph2, \
             tc.tile_pool(name="w2", bufs=3) as w2, \
             tc.tile_pool(name="ps2", bufs=2, space="PSUM") as ps2:

            yacc = [ph2.tile([128, TOK], F32, name=f"yacc{ct}")
                    for ct in range(2)]
            hlast = {p: ph2.tile([128, 32], F32, name=f"hl{p}")
                     for p in ("f", "b")}

            for p in ("f", "b"):
                d = br_w[p]
                row0 = 0 if p == "f" else 96
                for ck in range(NCK):
                    # proj chunk from DRAM (dt rows)
                    pjc = w2.tile([R, CK], F32, tag="pjc")
                    nc.sync.dma_start(
                        pjc[:], cc_out[row0:row0 + R,
                                       ck * CK:(ck + 1) * CK])
                    dts, ws = [], []
                    for ct in range(2):
                        pdt = ps2.tile([128, CK], F32, tag="pa")
                        nc.tensor.matmul(
                            pdt[:], d["dtwT"][:, ct * 128:(ct + 1) * 128],
                            pjc[:], start=True, stop=True)
                        e_ = w2.tile([128, CK], F32, tag="edt")
                        nc.scalar.activation(e_[:], pdt[:], AF.Exp,
                                             bias=d["dtb"][:, ct:ct + 1])
                        dt_ = w2.tile([128, CK], F32, tag=f"dt{ct}")
                        nc.scalar.activation(dt_[:], e_[:], AF.Ln, bias=1.0)
                        w_ = w2.tile([128, CK], F32, tag=f"w{ct}")
                        nc.vector.tensor_mul(
                            w_[:], dt_[:],
                            xc[p][ct][:, ck * CK:(ck + 1) * CK])
                        dts.append(dt_)
                        ws.append(w_)

                    yps = [ps2.tile([128, CK], F32, tag=f"py{ct}",
                                    name=f"py{ct}", bufs=1)
                           for ct in range(2)]
                    for cb in range(NCB):
                        ct, j = cb // 4, cb % 4
                        pdtr = ps2.tile([128, CK], F32, tag="pdtr")
                        nc.tensor.matmul(pdtr[:],
                                         sel_sb[:, j * 128:(j + 1) * 128],
                                         dts[ct][:], start=True, stop=True)
                        pwr = ps2.tile([128, CK], F32, tag="pwr")
                        nc.tensor.matmul(pwr[:],
                                         sel_sb[:, j * 128:(j + 1) * 128],
                                         ws[ct][:], start=True, stop=True)
                        for np_ in range(NP):
                            g = cb * NP + np_
                            brep = w2.tile([128, CK], F32, tag="brep")
                            nc.sync.dma_start(brep[:], bass.AP(
                                cc_out,
                                (row0 + 64 + np_ * 4) * TOK + ck * CK,
                                [[TOK, 4], [0, 32], [1, CK]]))
                            crep = w2.tile([128, CK], F32, tag="crep")
                            nc.sync.dma_start(crep[:], bass.AP(
                                cc_out,
                                (row0 + 80 + np_ * 4) * TOK + ck * CK,
                                [[TOK, 4], [0, 32], [1, CK]]))
                            dA = w2.tile([128, CK], F32, tag="dA")
                            nc.scalar.activation(
                                dA[:], pdtr[:], AF.Exp,
                                scale=d["acols"][:, g:g + 1])
                            dB = w2.tile([128, CK], F32, tag="dB")
                            nc.vector.tensor_mul(dB[:], pwr[:], brep[:])
                            h = w2.tile([128, CK], F32, tag="h")
                            init = (0.0 if ck % CPB == 0
                                    else hlast[p][:, g:g + 1])
                            nc.vector.tensor_tensor_scan(
                                h[:], dA[:], dB[:], init,
                                ALU.mult, ALU.add)
                            if ck % CPB != CPB - 1:
                                nc.sync.dma_start(hlast[p][:, g:g + 1],
                                                  h[:, CK - 1:CK])
                            hC = w2.tile([128, CK], F32, tag="hC")
                            if HC_ON_GPSIMD:
                                nc.gpsimd.tensor_mul(hC[:], h[:], crep[:])
                            else:
                                nc.vector.tensor_mul(hC[:], h[:], crep[:])
                            nc.tensor.matmul(
                                yps[ct][32 * j:32 * (j + 1), :],
                                sel32_sb[:], hC[:],
                                start=(np_ == 0), stop=(np_ == NP - 1),
                                tile_position=(0, 32 * j))
                    # gate
                    bb = ck // CPB
                    fck = bb * CPB + (CPB - 1 - ck % CPB)
                    for ct in range(2):
                        szt = w2.tile([128, CK], F32, tag="szg")
                        if p == "f":
                            nc.sync.dma_start(
                                szt[:],
                                sz_dram[ct * 128:(ct + 1) * 128,
                                        ck * CK:(ck + 1) * CK])
                        else:
                            nc.sync.dma_start(
                                szt[:],
                                sz_dram[ct * 128:(ct + 1) * 128,
                                        fck * CK:(fck + 1) * CK][:, ::-1])
                        y1 = w2.tile([128, CK], F32, tag="y1")
                        nc.vector.scalar_tensor_tensor(
                            y1[:], xc[p][ct][:, ck * CK:(ck + 1) * CK],
                            d["dvec"][:, ct:ct + 1], yps[ct][:],
                            ALU.mult, ALU.add)
                        if p == "f":
                            nc.vector.tensor_mul(
                                yacc[ct][:, ck * CK:(ck + 1) * CK],
                                y1[:], szt[:])
                        else:
                            y2 = w2.tile([128, CK], F32, tag="y2")
                            nc.vector.tensor_mul(y2[:], y1[:], szt[:])
                            dst = yacc[ct][:, fck * CK:(fck + 1) * CK]
                            nc.vector.tensor_add(dst[:], dst[:],
                                                 y2[:, ::-1])

            # out_proj partials
            for ck in range(NCK):
                for mt in range(8):
                    po = ps2.tile([128, CK], F32, tag="pa")
                    for ct in range(2):
                        nc.tensor.matmul(
                            po[:], wout_sb[:, ct * DM + mt * 128:
                                           ct * DM + (mt + 1) * 128],
                            yacc[ct][:, ck * CK:(ck + 1) * CK],
                            start=(ct == 0), stop=(ct == 1))
                    ob = w2.tile([128, CK], F32, tag="ob")
                    nc.scalar.copy(ob[:], po[:])
                    nc.sync.dma_start(
                        out_cc_in[mt * 128:(mt + 1) * 128,
                                  ck * CK:(ck + 1) * CK], ob[:])

        # ============ ReduceScatter ============
        nc.gpsimd.collective_compute(
            "ReduceScatter", ALU.add, replica_groups=GROUPS,
            ins=[out_cc_in.ap()], outs=[rs_out.ap()])
        nc.sync.dma_start(out_slice.ap(), rs_out.ap())

    nc.compile()
    return nc


def _prep_inputs(inputs):
    u = np.asarray(inputs["u"], np.float32)
    uT = np.ascontiguousarray(u.reshape(TOK, DM).T)

    sel_ = np.zeros((128, 4 * 128), np.float32)
    for j in range(4):
        for pp in range(128):
            sel_[j * 32 + pp % 32, j * 128 + pp] = 1.0
    sel32_ = np.zeros((128, 32), np.float32)
    for pp in range(128):
        sel32_[pp, pp % 32] = 1.0

    in_maps = []
    for core in range(NC):
        c0 = core * CH
        m = {"uT": uT, "sel": sel_, "sel32": sel32_}
        W = np.asarray(inputs["in_proj_w"], np.float32)
        m["winT"] = np.ascontiguousarray(
            np.concatenate([W[c0:c0 + CH], W[DI + c0:DI + c0 + CH]], 0).T)
        m["woutT"] = np.ascontiguousarray(
            np.asarray(inputs["out_proj_w"], np.float32)[:, c0:c0 + CH].T)

        for p, pref in (("f", "fwd_"), ("b", "bwd_")):
            cw = np.asarray(inputs[pref + "conv_w"],
                            np.float32)[c0:c0 + CH, 0, :]
            m[f"{p}cw"] = np.ascontiguousarray(cw)
            m[f"{p}cbias"] = np.ascontiguousarray(
                np.asarray(inputs[pref + "conv_b"],
                           np.float32)[c0:c0 + CH, None])
            xpT = np.asarray(inputs[pref + "x_proj_w"],
                             np.float32)[:, c0:c0 + CH].T
            xpt_pack = np.zeros((128, 2 * 96), np.float32)
            xpt_pack[:, 0:96] = xpT[0:128]
            xpt_pack[:, 96:192] = xpT[128:256]
            m[f"{p}xpT"] = xpt_pack
            m[f"{p}dtwT"] = np.ascontiguousarray(
                np.asarray(inputs[pref + "dt_w"], np.float32)[c0:c0 + CH].T)
            m[f"{p}dtb"] = np.ascontiguousarray(
                np.asarray(inputs[pref + "dt_b"],
                           np.float32)[c0:c0 + CH, None])
            A = -np.exp(np.asarray(inputs[pref + "A_log"],
                                   np.float32)[c0:c0 + CH])
            ac = np.zeros((128, 32), np.float32)
            for g in range(32):
                cb, np_ = g // NP, g % NP
                for pp in range(128):
                    ac[pp, g] = A[cb * 32 + pp % 32, np_ * 4 + pp // 32]
            m[f"{p}acols"] = ac
            m[f"{p}dvec"] = np.ascontiguousarray(
                np.asarray(inputs[pref + "D"], np.float32)[c0:c0 + CH, None])
        in_maps.append(m)
    return in_maps


def kernel(**inputs) -> np.ndarray:
    if "nc" not in _CACHE:
        _CACHE["nc"] = build_program()
    nc = _CACHE["nc"]
    in_maps = _prep_inputs(inputs)
    res = run_bass_kernel_spmd(nc, in_maps, list(range(NC)))
    out_full = np.concatenate(
        [np.asarray(res.results[i]["out_slice"]) for i in range(NC)], 0)
    y = out_full.reshape(DM, B, L).transpose(1, 2, 0)
    return np.ascontiguousarray(y).astype(np.float32)


if __name__ == "__main__":
    d = np.load('/root/problem/inputs.npz')
    inputs = {k: d[k] for k in d.files}
    got = kernel(**inputs)
    from ref_np import reference_np
    exp = reference_np(**inputs)
    err = np.abs(got - exp).max() / (np.abs(exp).max() + 1e-30)
    print("Relative error:", err)


# revision 8
# speedup vs baseline: 1.8648x; 1.8648x over previous
"""BiMamba block on 8 Trainium2 NeuronCores.

Sharding: d_inner (2048) split 8 ways -> 256 channels/core, both branches,
both batch elems per core. Cross-core collectives: AllReduce for the x_proj
partial sums (reduction over d_inner), ReduceScatter for the output
projection (reduction over d_inner, output row-sharded).

Core layout choices:
- tokens t = b*2048 + l along the free dim (4096 per core); scans split per
  batch at chunk boundaries.
- SSM scan runs per "group": 128 partitions = 4 n-values x 32 channels
  (p = ln*32 + lc). 8 channel-blocks (cb) x 4 n-quarters (np) = 32 groups.
- dt/w replication across n via PE selector matmuls (PSUM), dA = exp via
  ScalarE with per-partition scale A, dB/scan on VectorE
  (tensor_tensor_scan does the recurrence), y-reduce over n via col-tiled
  accumulating matmuls into one PSUM bank, gating on VectorE.
- bwd branch runs in flipped time via reversed access patterns (no copies).
"""
import sys, os
sys.path.insert(0, '/opt/trn_rl_repo')
os.environ.setdefault("JAX_PLATFORMS", "cpu")

import numpy as np
from contextlib import ExitStack

import concourse.bass as bass
import concourse.tile as tile
from concourse import bacc, mybir
from concourse.bass_utils import run_bass_kernel_spmd

F32 = mybir.dt.float32
BF16 = mybir.dt.bfloat16
AF = mybir.ActivationFunctionType
ALU = mybir.AluOpType

# problem sizes
B, L, DM, DI, N, R, KC = 2, 2048, 1024, 2048, 16, 64, 4
NC = 8
CH = DI // NC            # 256 channels per core
TOK = B * L              # 4096
CK = 512                 # token chunk
NCK = TOK // CK          # 8
CPB = L // CK            # chunks per batch = 4
NCB = CH // 32           # 8 channel-blocks of 32
NP = N // 4              # 4 n-quarters
PADL = L + 6             # per-batch padded cols: [3 zero][L data][3 zero]

# engine flags
HC_ON_GPSIMD = True

_CACHE = {}


def build_program():
    nc = bacc.Bacc("TRN2", target_bir_lowering=False, debug=False,
                   num_devices=NC)

    ext = {}
    def ein(name, shape):
        ext[name] = nc.dram_tensor(name, list(shape), F32,
                                   kind="ExternalInput")
        return ext[name]

    uT = ein("uT", (DM, TOK))
    winT = ein("winT", (DM, 2 * CH))
    woutT = ein("woutT", (CH, DM))
    sel = ein("sel", (128, 4 * 128))
    sel32 = ein("sel32", (128, 32))
    for p in ("f", "b"):
        ein(f"{p}cw", (CH, KC))
        ein(f"{p}cbias", (CH, 1))
        ein(f"{p}xpT", (128, 2 * 96))
        ein(f"{p}dtwT", (R, CH))
        ein(f"{p}dtb", (CH, 1))
        ein(f"{p}acols", (128, 32))
        ein(f"{p}dvec", (CH, 1))

    out_slice = nc.dram_tensor("out_slice", [DM // NC, TOK], F32,
                               kind="ExternalOutput")

    sz_dram = nc.dram_tensor("sz_dram", [CH, TOK], F32)
    cc_in = nc.dram_tensor("cc_in", [192, TOK], F32)
    cc_out = nc.dram_tensor("cc_out", [192, TOK], F32, addr_space="Shared")
    out_cc_in = nc.dram_tensor("out_cc_in", [DM, TOK], F32)
    rs_out = nc.dram_tensor("rs_out", [DM // NC, TOK], F32)

    GROUPS = [list(range(NC))]

    with tile.TileContext(nc) as tc, ExitStack() as ctx:
        # ---- persistent pools
        wp = ctx.enter_context(tc.tile_pool(name="wp", bufs=1))
        big = ctx.enter_context(tc.tile_pool(name="big", bufs=1))

        sel_sb = wp.tile([128, 4 * 128], F32)
        nc.sync.dma_start(sel_sb[:], sel[:])
        sel32_sb = wp.tile([128, 32], F32)
        nc.sync.dma_start(sel32_sb[:], sel32[:])
        wout_sb = wp.tile([128, 2 * DM], F32)
        for k in range(2):
            nc.sync.dma_start(wout_sb[:, k * DM:(k + 1) * DM],
                              woutT[k * 128:(k + 1) * 128, :])

        br_w = {}
        for p in ("f", "b"):
            d = {}
            d["cw"] = wp.tile([128, 2 * KC], F32, name=f"{p}cw_sb")
            for ct in range(2):
                nc.sync.dma_start(d["cw"][:, ct * KC:(ct + 1) * KC],
                                  ext[f"{p}cw"][ct * 128:(ct + 1) * 128, :])
            for nm in ("cbias", "dtb", "dvec"):
                t_ = wp.tile([128, 2], F32, name=f"{p}{nm}_sb")
                for ct in range(2):
                    nc.sync.dma_start(
                        t_[:, ct:ct + 1],
                        ext[f"{p}{nm}"][ct * 128:(ct + 1) * 128, :])
                d[nm] = t_
            d["xpT"] = wp.tile([128, 2 * 96], F32, name=f"{p}xpT_sb")
            nc.sync.dma_start(d["xpT"][:], ext[f"{p}xpT"][:])
            d["dtwT"] = wp.tile([R, CH], F32, name=f"{p}dtwT_sb")
            nc.sync.dma_start(d["dtwT"][:], ext[f"{p}dtwT"][:])
            d["acols"] = wp.tile([128, 32], F32, name=f"{p}acols_sb")
            nc.sync.dma_start(d["acols"][:], ext[f"{p}acols"][:])
            br_w[p] = d

        # persistent activations: xc per branch per ct
        xc = {p: [big.tile([128, TOK], F32, name=f"xc{p}{ct}")
                  for ct in range(2)] for p in ("f", "b")}

        # ============ phase 1: in_proj + conv + x_proj partials ============
        with tc.tile_pool(name="ph1", bufs=1) as ph1, \
             tc.tile_pool(name="w1", bufs=2) as w1, \
             tc.tile_pool(name="ps1", bufs=2, space="PSUM") as ps1:

            win_sb = ph1.tile([128, 8 * 512], F32)
            for k in range(8):
                nc.sync.dma_start(win_sb[:, k * 512:(k + 1) * 512],
                                  winT[k * 128:(k + 1) * 128, :])
            x_pad = [ph1.tile([128, B * PADL], F32, name=f"xpad{ct}")
                     for ct in range(2)]
            for ct in range(2):
                for bb in range(B):
                    nc.vector.memset(
                        x_pad[ct][:, bb * PADL:bb * PADL + 3], 0.0)
                    nc.vector.memset(
                        x_pad[ct][:, bb * PADL + 3 + L:(bb + 1) * PADL], 0.0)

            def dcol(ckk):
                bb = ckk // CPB
                return bb * PADL + 3 + (ckk % CPB) * CK

            # in_proj
            for ck in range(NCK):
                ut = w1.tile([128, 8 * CK], F32, tag="ut")
                for k in range(8):
                    nc.sync.dma_start(ut[:, k * CK:(k + 1) * CK],
                                      uT[k * 128:(k + 1) * 128,
                                         ck * CK:(ck + 1) * CK])
                for mt in range(4):
                    pin = ps1.tile([128, CK], F32, tag="pa")
                    for k in range(8):
                        nc.tensor.matmul(
                            pin[:], win_sb[:, k * 512 + mt * 128:
                                           k * 512 + (mt + 1) * 128],
                            ut[:, k * CK:(k + 1) * CK],
                            start=(k == 0), stop=(k == 7))
                    if mt < 2:
                        c0 = dcol(ck)
                        nc.scalar.copy(x_pad[mt][:, c0:c0 + CK], pin[:])
                    else:
                        ct = mt - 2
                        sg = w1.tile([128, CK], F32, tag="sg")
                        nc.scalar.activation(sg[:], pin[:], AF.Sigmoid)
                        szt = w1.tile([128, CK], F32, tag="szt")
                        nc.vector.tensor_mul(szt[:], pin[:], sg[:])
                        nc.sync.dma_start(
                            sz_dram[ct * 128:(ct + 1) * 128,
                                    ck * CK:(ck + 1) * CK], szt[:])

            # conv + silu + x_proj partials, both branches
            for p in ("f", "b"):
                d = br_w[p]
                for ck in range(NCK):
                    bb = ck // CPB
                    cc = ck % CPB
                    for ct in range(2):
                        acc = None
                        for k in range(4):
                            if p == "f":
                                c0 = dcol(ck)
                                src = x_pad[ct][:, c0 - 3 + k:
                                                c0 - 3 + k + CK]
                            else:
                                c_hi = bb * PADL + 3 + (L - 1) - cc * CK \
                                    + 3 - k
                                src = x_pad[ct][:, c_hi - CK + 1:
                                                c_hi + 1][:, ::-1]
                            if k == 0:
                                acc = w1.tile([128, CK], F32, tag="acc0")
                                nc.vector.tensor_scalar_mul(
                                    acc[:], src,
                                    d["cw"][:, ct * KC:ct * KC + 1])
                            else:
                                acc2 = w1.tile([128, CK], F32, tag=f"acc{k}")
                                nc.vector.scalar_tensor_tensor(
                                    acc2[:], src,
                                    d["cw"][:, ct * KC + k:ct * KC + k + 1],
                                    acc[:], ALU.mult, ALU.add)
                                acc = acc2
                        sg = w1.tile([128, CK], F32, tag="sgc")
                        nc.scalar.activation(sg[:], acc[:], AF.Sigmoid,
                                             bias=d["cbias"][:, ct:ct + 1])
                        nc.vector.scalar_tensor_tensor(
                            xc[p][ct][:, ck * CK:(ck + 1) * CK], acc[:],
                            d["cbias"][:, ct:ct + 1], sg[:],
                            ALU.add, ALU.mult)
                    pxp = ps1.tile([96, CK], F32, tag="pxp")
                    for ct in range(2):
                        nc.tensor.matmul(
                            pxp[:], d["xpT"][:, ct * 96:(ct + 1) * 96],
                            xc[p][ct][:, ck * CK:(ck + 1) * CK],
                            start=(ct == 0), stop=(ct == 1))
                    pj = w1.tile([96, CK], F32, tag="pj")
                    nc.scalar.copy(pj[:], pxp[:])
                    row0 = 0 if p == "f" else 96
                    nc.sync.dma_start(
                        cc_in[row0:row0 + 96, ck * CK:(ck + 1) * CK], pj[:])

        # ============ AllReduce x_proj partials ============
        nc.gpsimd.collective_compute(
            "AllReduce", ALU.add, replica_groups=GROUPS,
            ins=[cc_in.ap()], outs=[cc_out.ap()])

        # ============ phase 2: scan + gate + out_proj ============
        with tc.tile_pool(name="ph2", bufs=1) as ph2, \
             tc.tile_pool(name="w2", bufs=2) as w2, \
             tc.tile_pool(name="ps2", bufs=2, space="PSUM") as ps2:

            yacc = [ph2.tile([128, TOK], F32, name=f"yacc{ct}")
                    for ct in range(2)]
            hlast = {p: ph2.tile([128, 32], F32, name=f"hl{p}")
                     for p in ("f", "b")}

            for p in ("f", "b"):
                d = br_w[p]
                row0 = 0 if p == "f" else 96
                for ck in range(NCK):
                    # proj chunk from DRAM (dt rows)
                    pjc = w2.tile([R, CK], F32, tag="pjc")
                    nc.sync.dma_start(
                        pjc[:], cc_out[row0:row0 + R,
                                       ck * CK:(ck + 1) * CK])
                    dts, ws = [], []
                    for ct in range(2):
                        pdt = ps2.tile([128, CK], F32, tag="pa")
                        nc.tensor.matmul(
                            pdt[:], d["dtwT"][:, ct * 128:(ct + 1) * 128],
                            pjc[:], start=True, stop=True)
                        e_ = w2.tile([128, CK], F32, tag="edt")
                        nc.scalar.activation(e_[:], pdt[:], AF.Exp,
                                             bias=d["dtb"][:, ct:ct + 1])
                        dt_ = w2.tile([128, CK], F32, tag=f"dt{ct}")
                        nc.scalar.activation(dt_[:], e_[:], AF.Ln, bias=1.0)
                        w_ = w2.tile([128, CK], F32, tag=f"w{ct}")
                        nc.vector.tensor_mul(
                            w_[:], dt_[:],
                            xc[p][ct][:, ck * CK:(ck + 1) * CK])
                        dts.append(dt_)
                        ws.append(w_)

                    yps = [ps2.tile([128, CK], F32, tag=f"py{ct}",
                                    name=f"py{ct}", bufs=1)
                           for ct in range(2)]
                    brep = w2.tile([128, NP * CK], F32, tag="brep")
                    crep = w2.tile([128, NP * CK], F32, tag="crep")
                    for np_ in range(NP):
                        nc.sync.dma_start(
                            brep[:, np_ * CK:(np_ + 1) * CK], bass.AP(
                                cc_out,
                                (row0 + 64 + np_ * 4) * TOK + ck * CK,
                                [[TOK, 4], [0, 32], [1, CK]]))
                        nc.sync.dma_start(
                            crep[:, np_ * CK:(np_ + 1) * CK], bass.AP(
                                cc_out,
                                (row0 + 80 + np_ * 4) * TOK + ck * CK,
                                [[TOK, 4], [0, 32], [1, CK]]))
                    for cb in range(NCB):
                        ct, j = cb // 4, cb % 4
                        pdtr = ps2.tile([128, CK], F32, tag="pdtr")
                        nc.tensor.matmul(pdtr[:],
                                         sel_sb[:, j * 128:(j + 1) * 128],
                                         dts[ct][:], start=True, stop=True)
                        pwr = ps2.tile([128, CK], F32, tag="pwr")
                        nc.tensor.matmul(pwr[:],
                                         sel_sb[:, j * 128:(j + 1) * 128],
                                         ws[ct][:], start=True, stop=True)
                        for np_ in range(NP):
                            g = cb * NP + np_
                            bslc = brep[:, np_ * CK:(np_ + 1) * CK]
                            cslc = crep[:, np_ * CK:(np_ + 1) * CK]
                            dA = w2.tile([128, CK], F32, tag="dA")
                            nc.scalar.activation(
                                dA[:], pdtr[:], AF.Exp,
                                scale=d["acols"][:, g:g + 1])
                            dB = w2.tile([128, CK], F32, tag="dB")
                            nc.vector.tensor_mul(dB[:], pwr[:], bslc)
                            h = w2.tile([128, CK], F32, tag="h")
                            init = (0.0 if ck % CPB == 0
                                    else hlast[p][:, g:g + 1])
                            nc.vector.tensor_tensor_scan(
                                h[:], dA[:], dB[:], init,
                                ALU.mult, ALU.add)
                            if ck % CPB != CPB - 1:
                                nc.scalar.copy(hlast[p][:, g:g + 1],
                                               h[:, CK - 1:CK])
                            hC = w2.tile([128, CK], F32, tag="hC")
                            if HC_ON_GPSIMD:
                                nc.gpsimd.tensor_mul(hC[:], h[:], cslc)
                            else:
                                nc.vector.tensor_mul(hC[:], h[:], cslc)
                            nc.tensor.matmul(
                                yps[ct][32 * j:32 * (j + 1), :],
                                sel32_sb[:], hC[:],
                                start=(np_ == 0), stop=(np_ == NP - 1),
                                tile_position=(0, 32 * j))
                    # gate
                    bb = ck // CPB
                    fck = bb * CPB + (CPB - 1 - ck % CPB)
                    for ct in range(2):
                        szt = w2.tile([128, CK], F32, tag="szg")
                        if p == "f":
                            nc.sync.dma_start(
                                szt[:],
                                sz_dram[ct * 128:(ct + 1) * 128,
                                        ck * CK:(ck + 1) * CK])
                        else:
                            nc.sync.dma_start(
                                szt[:],
                                sz_dram[ct * 128:(ct + 1) * 128,
                                        fck * CK:(fck + 1) * CK][:, ::-1])
                        y1 = w2.tile([128, CK], F32, tag="y1")
                        nc.vector.scalar_tensor_tensor(
                            y1[:], xc[p][ct][:, ck * CK:(ck + 1) * CK],
                            d["dvec"][:, ct:ct + 1], yps[ct][:],
                            ALU.mult, ALU.add)
                        if p == "f":
                            nc.vector.tensor_mul(
                                yacc[ct][:, ck * CK:(ck + 1) * CK],
                                y1[:], szt[:])
                        else:
                            y2 = w2.tile([128, CK], F32, tag="y2")
                            nc.vector.tensor_mul(y2[:], y1[:], szt[:])
                            dst = yacc[ct][:, fck * CK:(fck + 1) * CK]
                            nc.vector.tensor_add(dst[:], dst[:],
                                                 y2[:, ::-1])

            # out_proj partials
            for ck in range(NCK):
                for mt in range(8):
                    po = ps2.tile([128, CK], F32, tag="pa")
                    for ct in range(2):
                        nc.tensor.matmul(
                            po[:], wout_sb[:, ct * DM + mt * 128:
                                           ct * DM + (mt + 1) * 128],
                            yacc[ct][:, ck * CK:(ck + 1) * CK],
                            start=(ct == 0), stop=(ct == 1))
                    ob = w2.tile([128, CK], F32, tag="ob")
                    nc.scalar.copy(ob[:], po[:])
                    nc.sync.dma_start(
                        out_cc_in[mt * 128:(mt + 1) * 128,
                                  ck * CK:(ck + 1) * CK], ob[:])

        # ============ ReduceScatter ============
        nc.gpsimd.collective_compute(
            "ReduceScatter", ALU.add, replica_groups=GROUPS,
            ins=[out_cc_in.ap()], outs=[rs_out.ap()])
        nc.sync.dma_start(out_slice.ap(), rs_out.ap())

    nc.compile()
    return nc


def _prep_inputs(inputs):
    u = np.asarray(inputs["u"], np.float32)
    uT = np.ascontiguousarray(u.reshape(TOK, DM).T)

    sel_ = np.zeros((128, 4 * 128), np.float32)
    for j in range(4):
        for pp in range(128):
            sel_[j * 32 + pp % 32, j * 128 + pp] = 1.0
    sel32_ = np.zeros((128, 32), np.float32)
    for pp in range(128):
        sel32_[pp, pp % 32] = 1.0

    in_maps = []
    for core in range(NC):
        c0 = core * CH
        m = {"uT": uT, "sel": sel_, "sel32": sel32_}
        W = np.asarray(inputs["in_proj_w"], np.float32)
        m["winT"] = np.ascontiguousarray(
            np.concatenate([W[c0:c0 + CH], W[DI + c0:DI + c0 + CH]], 0).T)
        m["woutT"] = np.ascontiguousarray(
            np.asarray(inputs["out_proj_w"], np.float32)[:, c0:c0 + CH].T)

        for p, pref in (("f", "fwd_"), ("b", "bwd_")):
            cw = np.asarray(inputs[pref + "conv_w"],
                            np.float32)[c0:c0 + CH, 0, :]
            m[f"{p}cw"] = np.ascontiguousarray(cw)
            m[f"{p}cbias"] = np.ascontiguousarray(
                np.asarray(inputs[pref + "conv_b"],
                           np.float32)[c0:c0 + CH, None])
            xpT = np.asarray(inputs[pref + "x_proj_w"],
                             np.float32)[:, c0:c0 + CH].T
            xpt_pack = np.zeros((128, 2 * 96), np.float32)
            xpt_pack[:, 0:96] = xpT[0:128]
            xpt_pack[:, 96:192] = xpT[128:256]
            m[f"{p}xpT"] = xpt_pack
            m[f"{p}dtwT"] = np.ascontiguousarray(
                np.asarray(inputs[pref + "dt_w"], np.float32)[c0:c0 + CH].T)
            m[f"{p}dtb"] = np.ascontiguousarray(
                np.asarray(inputs[pref + "dt_b"],
                           np.float32)[c0:c0 + CH, None])
            A = -np.exp(np.asarray(inputs[pref + "A_log"],
                                   np.float32)[c0:c0 + CH])
            ac = np.zeros((128, 32), np.float32)
            for g in range(32):
                cb, np_ = g // NP, g % NP
                for pp in range(128):
                    ac[pp, g] = A[cb * 32 + pp % 32, np_ * 4 + pp // 32]
            m[f"{p}acols"] = ac
            m[f"{p}dvec"] = np.ascontiguousarray(
                np.asarray(inputs[pref + "D"], np.float32)[c0:c0 + CH, None])
        in_maps.append(m)
    return in_maps


def kernel(**inputs) -> np.ndarray:
    if "nc" not in _CACHE:
        _CACHE["nc"] = build_program()
    nc = _CACHE["nc"]
    in_maps = _prep_inputs(inputs)
    res = run_bass_kernel_spmd(nc, in_maps, list(range(NC)))
    out_full = np.concatenate(
        [np.asarray(res.results[i]["out_slice"]) for i in range(NC)], 0)
    y = out_full.reshape(DM, B, L).transpose(1, 2, 0)
    return np.ascontiguousarray(y).astype(np.float32)


if __name__ == "__main__":
    d = np.load('/root/problem/inputs.npz')
    inputs = {k: d[k] for k in d.files}
    got = kernel(**inputs)
    from ref_np import reference_np
    exp = reference_np(**inputs)
    err = np.abs(got - exp).max() / (np.abs(exp).max() + 1e-30)
    print("Relative error:", err)
